# revision 1
# baseline (speedup 1.0000x reference)
"""Ernie4 MoE (T=2048, H=1024, E=64 top-6, I=512 + shared SwiGLU MLP) on 8 Trainium2 cores.

Strategy: expert parallelism. Each core owns 8 experts (weights sharded on host),
replicates the router gate, and tensor-parallels the shared MLP (SI split 8 ways).
On device each core:
  1. computes gate logits (fp32), sigmoid scores, top-6 selection and renormalized
     combine weights for all 64 experts,
  2. compacts, per local expert, the list of routed token ids with the gpsimd
     sparse_gather ucode instruction (capacity 384/expert),
  3. gathers routed token activations by indirect DMA, runs the expert SwiGLU FFN
     on the PE array (fp32r), scales by the combine weight, and scatter-ADDS the
     result into the output with indirect DMA (CCE add),
  4. adds its shared-MLP slice partial.
The host sums the 8 per-core partial outputs (the "all-reduce" of the TP/EP plan).
"""

import numpy as np

T, H, E, K, I, SI = 2048, 1024, 64, 6, 512, 1024
NCORE = 8
EC = E // NCORE          # experts per core
C = 384                  # token capacity per expert (max observed count + margin)
CCH = C // 128           # slot chunks per expert
KC = H // 128            # hidden-dim 128-chunks
ICN = I // 128           # expert-intermediate 128-chunks
TCN = T // 128           # token 128-chunks
SIC = SI // NCORE        # shared-intermediate slice per core
BIG = 1e30

_CACHE = {}


def _build():
    import concourse.bass as bass
    import concourse.tile as tile
    from concourse import bacc, mybir
    from concourse.bass import IndirectOffsetOnAxis

    f32 = mybir.dt.float32
    f32r = mybir.dt.float32r
    i32 = mybir.dt.int32
    u32 = mybir.dt.uint32
    AF = mybir.ActivationFunctionType
    OP = mybir.AluOpType
    AX = mybir.AxisListType

    def r(ap):
        return ap.bitcast(f32r)

    nc = bacc.Bacc("TRN2", target_bir_lowering=False, debug=False,
                   enable_asserts=False, num_devices=NCORE)

    xT = nc.dram_tensor("xT", [H, T], f32, kind="ExternalInput").ap()
    xp = nc.dram_tensor("xp", [T + 1, H], f32, kind="ExternalInput").ap()
    gwT = nc.dram_tensor("gwT", [H, E], f32, kind="ExternalInput").ap()
    biasr = nc.dram_tensor("biasr", [128, E], f32, kind="ExternalInput").ap()
    wg = nc.dram_tensor("wg", [EC, H, I], f32, kind="ExternalInput").ap()
    wu = nc.dram_tensor("wu", [EC, H, I], f32, kind="ExternalInput").ap()
    wd = nc.dram_tensor("wd", [EC, I, H], f32, kind="ExternalInput").ap()
    wsg = nc.dram_tensor("wsg", [H, SIC], f32, kind="ExternalInput").ap()
    wsu = nc.dram_tensor("wsu", [H, SIC], f32, kind="ExternalInput").ap()
    wsd = nc.dram_tensor("wsd", [SIC, H], f32, kind="ExternalInput").ap()
    tokp1 = nc.dram_tensor("tokp1", [16, T // 16], f32, kind="ExternalInput").ap()
    pos24 = nc.dram_tensor("pos24", [16, C // 16], f32, kind="ExternalInput").ap()
    ident = nc.dram_tensor("ident", [128, 128], f32, kind="ExternalInput").ap()
    outp = nc.dram_tensor("outp", [T + 1, H], f32, kind="ExternalOutput").ap()

    cmb_d = nc.dram_tensor("cmb_d", [T + 1, 64], f32, kind="Internal").ap()
    import os as _os
    _selkind = "ExternalOutput" if _os.environ.get("KDEBUG_SEL") else "Internal"
    sel_d = nc.dram_tensor("sel_d", [T, EC], f32, kind=_selkind).ap()
    if _os.environ.get("KDEBUG_SEL"):
        scores_d = nc.dram_tensor("scores_d", [128, TCN, E], f32, kind="ExternalOutput").ap()
    else:
        scores_d = None

    with tile.TileContext(nc) as tc:
        with (
            tc.tile_pool(name="consts", bufs=1) as consts,
            tc.tile_pool(name="wpool", bufs=2) as wpool,
            tc.tile_pool(name="etmp", bufs=2) as etmp,
            tc.tile_pool(name="smalls", bufs=1) as smalls,
            tc.tile_pool(name="ps_small", bufs=4, space="PSUM") as ps_s,
            tc.tile_pool(name="ps_big", bufs=2, space="PSUM") as ps_b,
        ):
            # ---- constants ----
            ident_sb = consts.tile([128, 128], f32)
            nc.sync.dma_start(ident_sb[:], ident)
            tokp1_sb = consts.tile([16, T // 16], f32)
            nc.sync.dma_start(tokp1_sb[:], tokp1)
            bias_sb = consts.tile([128, E], f32)
            nc.sync.dma_start(bias_sb[:], biasr)
            pos_sb = consts.tile([16, C // 16], f32)
            nc.sync.dma_start(pos_sb[:], pos24)
            ones128 = consts.tile([128, 1], f32)
            nc.vector.memset(ones128[:], 1.0)
            ones16 = consts.tile([1, 16], f32)
            nc.vector.memset(ones16[:], 1.0)

            # per-expert wrapped token-index tiles (live through the whole kernel)
            idx128 = [smalls.tile([128, C // 16], mybir.dt.int16, tag=f"idx{e}",
                                  name=f"idx128_{e}") for e in range(EC)]

            # ---- expert weight streaming (separate HWDGE FIFO: scalar engine) ----
            wg_sbs, wu_sbs, wd_sbs = [], [], []
            for e in range(EC):
                wg_sb = wpool.tile([128, KC, I], f32r, tag="wg")
                nc.scalar.dma_start(wg_sb[:], wg[e].rearrange("(kc p) i -> p kc i", p=128).bitcast(f32r))
                wu_sb = wpool.tile([128, KC, I], f32r, tag="wu")
                nc.scalar.dma_start(wu_sb[:], wu[e].rearrange("(kc p) i -> p kc i", p=128).bitcast(f32r))
                wd_sb = wpool.tile([128, ICN, H], f32r, tag="wd")
                nc.scalar.dma_start(wd_sb[:], wd[e].rearrange("(ic p) h -> p ic h", p=128).bitcast(f32r))
                wg_sbs.append(wg_sb); wu_sbs.append(wu_sb); wd_sbs.append(wd_sb)

            with (
                tc.tile_pool(name="ph1", bufs=2) as ph1,
                tc.tile_pool(name="route", bufs=1) as route,
            ):
                # gate weights first: they gate the logits critical path
                gwT_sb = ph1.tile([128, KC, E], f32, tag="gwT")
                nc.sync.dma_start(gwT_sb[:], gwT.rearrange("(kc p) e -> p kc e", p=128))
                wsg_sb = ph1.tile([128, KC, SIC], f32r, tag="wsg")
                nc.sync.dma_start(wsg_sb[:], wsg.rearrange("(kc p) s -> p kc s", p=128).bitcast(f32r))
                wsu_sb = ph1.tile([128, KC, SIC], f32r, tag="wsu")
                nc.sync.dma_start(wsu_sb[:], wsu.rearrange("(kc p) s -> p kc s", p=128).bitcast(f32r))
                wsd_sb = ph1.tile([128, H], f32r, tag="wsd")
                nc.sync.dma_start(wsd_sb[:], wsd.bitcast(f32r))

                scores = route.tile([128, TCN, E], f32, tag="scores")
                a_s = route.tile([128, 8, 256], f32r, tag="a_s")

                # ---- phase 1a: all gate logits first (exact fp32) so routing +
                # compaction overlap the shared-expert compute that follows ----
                for sl in range(TCN // 2):
                    xtl = ph1.tile([128, KC, 256], f32, tag="xtl")
                    nc.sync.dma_start(
                        xtl[:], xT.rearrange("(kc p) t -> p kc t", p=128)[:, :, sl * 256:(sl + 1) * 256])
                    for j in range(2):
                        tci = sl * 2 + j
                        pl = ps_s.tile([128, 512], f32, tag="mm_small")
                        for kc in range(KC):
                            nc.tensor.matmul(pl[:, :E], xtl[:, kc, j * 128:(j + 1) * 128],
                                             gwT_sb[:, kc, :], start=(kc == 0), stop=(kc == KC - 1))
                        nc.scalar.activation(scores[:, tci, :], pl[:, :E], AF.Sigmoid)
                # ---- phase 1b: shared gate/up (fp32r) ----
                for s in range(8):  # 256-token slabs of xT
                    xts = ph1.tile([128, KC, 256], f32r, tag="xts")
                    nc.sync.dma_start(
                        xts[:], xT.rearrange("(kc p) t -> p kc t", p=128)[:, :, s * 256:(s + 1) * 256].bitcast(f32r))
                    pg = ps_s.tile([128, 512], f32, tag="mm_small")
                    pu = ps_s.tile([128, 512], f32, tag="mm_small")
                    for kc in range(KC):
                        nc.tensor.matmul(pg[:, :256], wsg_sb[:, kc, :], xts[:, kc, :],
                                         start=(kc == 0), stop=(kc == KC - 1))
                    for kc in range(KC):
                        nc.tensor.matmul(pu[:, :256], wsu_sb[:, kc, :], xts[:, kc, :],
                                         start=(kc == 0), stop=(kc == KC - 1))
                    sg_t = route.tile([128, 256], f32, tag="sgt")
                    nc.scalar.activation(sg_t[:], pg[:, :256], AF.Sigmoid)
                    gu_t = route.tile([128, 256], f32, tag="gut")
                    nc.vector.tensor_tensor(gu_t[:], sg_t[:], pg[:, :256], op=OP.mult)
                    nc.vector.tensor_tensor(a_s[:, s, :], gu_t[:], pu[:, :256], op=OP.mult)

                # ---- phase 3a: shared down-proj, dense write of partial out ----
                for tci in range(TCN):
                    s, j = tci // 2, tci % 2
                    py = ps_b.tile([128, H], f32, tag="mm_big")
                    for nh in range(2):
                        nc.tensor.matmul(py[:, nh * 512:(nh + 1) * 512],
                                         a_s[:, s, j * 128:(j + 1) * 128],
                                         wsd_sb[:, nh * 512:(nh + 1) * 512],
                                         start=True, stop=True)
                    ysh = route.tile([128, H], f32, tag="ysh", bufs=2)
                    nc.scalar.activation(ysh[:, 0:512], py[:, 0:512], AF.Copy)
                    nc.vector.tensor_copy(ysh[:, 512:1024], py[:, 512:1024])
                    nc.sync.dma_start(
                        outp[0:T].rearrange("(tc p) h -> p tc h", p=128)[:, tci, :], ysh[:])

                if scores_d is not None:
                    nc.sync.dma_start(scores_d, scores[:])
                # ---- phase 2: routing (DVE) ----
                work_t = [route.tile([128, TCN, E], f32, tag=f"work{i}", name=f"work{i}")
                          for i in range(2)]
                nc.vector.tensor_tensor(
                    work_t[0][:], scores[:],
                    bias_sb[:, None, :].to_broadcast([128, TCN, E]), op=OP.add)
                wsrc = work_t[0]
                for k in range(K):
                    m = route.tile([128, TCN], f32, tag=f"m{k % 2}")
                    nc.vector.reduce_max(m[:], wsrc[:], axis=AX.X)
                    eq = route.tile([128, TCN, E], f32, tag="eq")
                    nc.vector.tensor_tensor(
                        eq[:], wsrc[:], m[:, :, None].to_broadcast([128, TCN, E]),
                        op=OP.is_equal)
                    wdst = work_t[(k + 1) % 2] if k < K - 1 else work_t[0]
                    nc.vector.scalar_tensor_tensor(
                        wdst[:], eq[:], -BIG, wsrc[:], op0=OP.mult, op1=OP.add)
                    wsrc = wdst
                sel = route.tile([128, TCN, E], f32, tag="eq")
                nc.vector.tensor_scalar(sel[:], wsrc[:], -BIG / 2, None, op0=OP.is_lt)
                selprod = route.tile([128, TCN, E], f32, tag="work1")
                nc.vector.tensor_tensor(selprod[:], scores[:], sel[:], op=OP.mult)
                denom = route.tile([128, TCN], f32, tag="denom")
                nc.vector.tensor_reduce(denom[:], selprod[:], axis=AX.X, op=OP.add)
                rec = route.tile([128, TCN], f32, tag="rec")
                nc.vector.reciprocal(rec[:], denom[:])
                cmb8 = route.tile([128, TCN, EC], f32, tag="cmb8")
                nc.vector.tensor_tensor(
                    cmb8[:], selprod[:, :, 0:EC],
                    rec[:, :, None].to_broadcast([128, TCN, EC]), op=OP.mult)

                # roundtrip through DRAM to re-wrap layouts (full 64-wide rows,
                # zero-padded, so the 256B-row gating gather reads defined data)
                cmbw = route.tile([128, TCN, 64], f32, tag="work1")
                nc.vector.memset(cmbw[:], 0.0)
                nc.vector.tensor_copy(cmbw[:, :, 0:EC], cmb8[:])
                nc.sync.dma_start(
                    cmb_d[0:T].rearrange("(tc p) e -> p tc e", p=128), cmbw[:])
                zrow = route.tile([1, 64], f32, tag="zrow")
                nc.vector.memset(zrow[:], 0.0)
                nc.sync.dma_start(cmb_d[T:T + 1, :], zrow[:])
                nc.sync.dma_start(sel_d.rearrange("(tc p) e -> p tc e", p=128),
                                  sel[:, :, 0:EC])
                sel16 = route.tile([16, EC, T // 16], f32, tag="sel16")
                nc.sync.dma_start(sel16[:], sel_d.rearrange("(f q) e -> q e f", q=16))

                # per-expert routed counts: ones^T @ sel8 (PE), then reduce + bcast
                pc = ps_s.tile([1, 512], f32, tag="mm_small", name="pc")
                nc.tensor.matmul(pc[0:1, 0:128], ones128[:],
                                 sel[:, :, 0:EC].rearrange("p t e -> p e t"),
                                 start=True, stop=True)
                counts = route.tile([1, EC], f32, tag="counts")
                nc.vector.tensor_reduce(counts[:], pc[0:1, 0:128].rearrange(
                    "p (e t) -> p e t", e=EC), axis=AX.X, op=OP.add)
                pnf = ps_s.tile([16, 512], f32, tag="mm_small", name="pnf")
                nc.tensor.matmul(pnf[:, 0:EC], ones16[:], counts[:],
                                 start=True, stop=True)
                nf16 = route.tile([16, EC], f32, tag="nf16")
                nc.vector.tensor_copy(nf16[:], pnf[:, 0:EC])

                # masked token values in wrapped layout
                nc.vector.tensor_tensor(
                    sel16[:], sel16[:],
                    tokp1_sb[:, None, :].to_broadcast([16, EC, T // 16]), op=OP.mult)
                nc.vector.tensor_scalar_sub(sel16[:], sel16[:], 1.0)

                # ---- compaction: per-expert routed token lists ----
                nfs = route.tile([1, EC], u32, tag="nfs")
                for e in range(EC):
                    idxf = route.tile([16, C // 16], f32, tag=f"idxf{e % 2}")
                    nc.gpsimd.sparse_gather(idxf[:], sel16[:, e, :],
                                            num_found=nfs[0:1, e:e + 1])
                    # keep = position < count; squash the garbage tail to token T (trash row)
                    keep = route.tile([16, C // 16], f32, tag=f"keep{e % 2}")
                    nc.vector.tensor_scalar(keep[:], pos_sb[:], nf16[:, e:e + 1], None,
                                            op0=OP.is_lt)
                    k32 = route.tile([16, C // 16], i32, tag=f"k32{e % 2}")
                    nc.vector.tensor_copy(k32[:], keep[:])
                    km = route.tile([16, C // 16], i32, tag=f"km{e % 2}")
                    nc.vector.tensor_scalar_mul(km[:], k32[:], -1)
                    bits = route.tile([16, C // 16], i32, tag=f"bits{e % 2}")
                    nc.vector.tensor_tensor(bits[:], idxf[:].bitcast(i32), km[:],
                                            op=OP.bitwise_and)
                    km1 = route.tile([16, C // 16], f32, tag=f"km1{e % 2}")
                    nc.vector.tensor_scalar_sub(km1[:], keep[:], 1.0)
                    idxn = route.tile([16, C // 16], f32, tag=f"idxn{e % 2}")
                    nc.vector.scalar_tensor_tensor(idxn[:], km1[:], -float(T),
                                                   bits[:].bitcast(f32),
                                                   op0=OP.mult, op1=OP.add)
                    nc.vector.tensor_copy(idx128[e][0:16, :], idxn[:])
                    nc.sync.dma_start(idx128[e][16:32, :], idx128[e][0:16, :])
                    nc.sync.dma_start(idx128[e][32:64, :], idx128[e][0:32, :])
                    nc.sync.dma_start(idx128[e][64:128, :], idx128[e][0:64, :])

            # ---- phase 4: expert loop (software-pipelined: expert e+1's
            # gathers are issued on the Pool queue BEFORE expert e's scatter,
            # so the DMA prefetch is never head-of-line blocked) ----
            with tc.tile_pool(name="xpool", bufs=2) as xpool:
                xgs, cgs = {}, {}

                def emit_gathers(e):
                    xg = xpool.tile([128, CCH, H], f32, tag="xg", name=f"xg{e}")
                    nc.gpsimd.dma_gather(xg[:], xp, idx128[e][:], C, C, H)
                    cg = xpool.tile([128, CCH, 64], f32, tag="cg", name=f"cg{e}")
                    nc.gpsimd.dma_gather(cg[:], cmb_d, idx128[e][:], C, C, 64)
                    xgs[e], cgs[e] = xg, cg

                emit_gathers(0)
                for e in range(EC):
                    if e + 1 < EC:
                        emit_gathers(e + 1)
                    wg_sb, wu_sb, wd_sb = wg_sbs[e], wu_sbs[e], wd_sbs[e]
                    xg, cg = xgs.pop(e), cgs.pop(e)

                    xeT = xpool.tile([128, KC, C], f32r, tag="xeT")
                    for cc in range(CCH):
                        for hc in range(KC):
                            pt = ps_s.tile([128, 512], f32, tag="mm_small")
                            nc.tensor.transpose(pt[:, :128], xg[:, cc, hc * 128:(hc + 1) * 128],
                                                ident_sb[:])
                            nc.vector.tensor_copy(xeT[:, hc, cc * 128:(cc + 1) * 128],
                                                  pt[:, :128])

                    aT = xpool.tile([128, ICN, C], f32r, tag="aT")
                    for ic in range(ICN):
                        pg = ps_s.tile([128, 512], f32, tag="mm_small")
                        pu = ps_s.tile([128, 512], f32, tag="mm_small")
                        for kc in range(KC):
                            nc.tensor.matmul(pg[:, :C], wg_sb[:, kc, ic * 128:(ic + 1) * 128],
                                             xeT[:, kc, :], start=(kc == 0), stop=(kc == KC - 1))
                        for kc in range(KC):
                            nc.tensor.matmul(pu[:, :C], wu_sb[:, kc, ic * 128:(ic + 1) * 128],
                                             xeT[:, kc, :], start=(kc == 0), stop=(kc == KC - 1))
                        sg_t = etmp.tile([128, C], f32, tag="esilu")
                        nc.scalar.activation(sg_t[:], pg[:, :C], AF.Sigmoid)
                        gu_t = etmp.tile([128, C], f32, tag="egu")
                        nc.vector.tensor_tensor(gu_t[:], sg_t[:], pg[:, :C], op=OP.mult)
                        nc.vector.tensor_tensor(aT[:, ic, :], gu_t[:], pu[:, :C], op=OP.mult)

                    y_sb = xpool.tile([128, CCH, H], f32, tag="ysb")
                    for cc in range(CCH):
                        py = ps_b.tile([128, H], f32, tag="mm_big")
                        for ic in range(ICN):
                            for nh in range(2):
                                nc.tensor.matmul(py[:, nh * 512:(nh + 1) * 512],
                                                 aT[:, ic, cc * 128:(cc + 1) * 128],
                                                 wd_sb[:, ic, nh * 512:(nh + 1) * 512],
                                                 start=(ic == 0), stop=(ic == ICN - 1))
                        nc.scalar.activation(y_sb[:, cc, :], py[:], AF.Copy,
                                             scale=cg[:, cc, e:e + 1])
                    nc.gpsimd.dma_scatter_add(outp, y_sb[:], idx128[e][:], C, C, H)

    nc.compile()
    return nc


def _prep_in_maps(inputs):
    x = np.ascontiguousarray(inputs["hidden_states"], dtype=np.float32)
    gate_w = np.asarray(inputs["gate_w"], dtype=np.float32)
    gate_bias = np.asarray(inputs["gate_bias"], dtype=np.float32)
    w_gate = np.asarray(inputs["w_gate"], dtype=np.float32)
    w_up = np.asarray(inputs["w_up"], dtype=np.float32)
    w_down = np.asarray(inputs["w_down"], dtype=np.float32)
    ws_gate = np.asarray(inputs["ws_gate"], dtype=np.float32)
    ws_up = np.asarray(inputs["ws_up"], dtype=np.float32)
    ws_down = np.asarray(inputs["ws_down"], dtype=np.float32)

    xTc = np.ascontiguousarray(x.T)
    xpv = np.vstack([x, np.zeros((1, H), np.float32)])
    tokp1 = (np.arange(16)[:, None] + 16 * np.arange(T // 16)[None, :] + 1).astype(np.float32)
    pos24 = (np.arange(16)[:, None] + 16 * np.arange(C // 16)[None, :]).astype(np.float32)
    ident = np.eye(128, dtype=np.float32)

    in_maps = []
    for c in range(NCORE):
        loc = list(range(c * EC, (c + 1) * EC))
        perm = loc + [e for e in range(E) if e not in loc]
        in_maps.append({
            "xp": xpv,
            "xT": xTc,
            "gwT": np.ascontiguousarray(gate_w[perm].T),
            "biasr": np.ascontiguousarray(
                np.broadcast_to(gate_bias[0, perm], (128, E))).astype(np.float32),
            "wg": np.ascontiguousarray(w_gate[loc]),
            "wu": np.ascontiguousarray(w_up[loc]),
            "wd": np.ascontiguousarray(w_down[loc]),
            "wsg": np.ascontiguousarray(ws_gate[:, c * SIC:(c + 1) * SIC]),
            "wsu": np.ascontiguousarray(ws_up[:, c * SIC:(c + 1) * SIC]),
            "wsd": np.ascontiguousarray(ws_down[c * SIC:(c + 1) * SIC, :]),
            "tokp1": tokp1,
            "pos24": pos24,
            "ident": ident,
        })
    return in_maps


def get_nc():
    if "nc" not in _CACHE:
        _CACHE["nc"] = _build()
    return _CACHE["nc"]


def kernel(**inputs) -> np.ndarray:
    from concourse import bass_utils
    nc = get_nc()
    in_maps = _prep_in_maps(inputs)
    res = bass_utils.run_bass_kernel_spmd(nc, in_maps, core_ids=list(range(NCORE)))
    acc = np.zeros((T, H), dtype=np.float64)
    for c in range(NCORE):
        acc += res.results[c]["outp"][0:T].astype(np.float64)
    return acc.astype(np.float32)



# revision 6
# speedup vs baseline: 1.5659x; 1.5659x over previous
"""Ernie4 MoE (T=2048, H=1024, E=64 top-6, I=512 + shared SwiGLU MLP) on 8 Trainium2 cores.

Strategy: expert parallelism, bf16 compute with an exact-fp32 router.
Each core owns 8 experts (host assigns experts to (core, slot) by routed-count
snake order so per-slot capacities [384, 256x7] cover the biggest experts),
replicates the router gate, and tensor-parallels the shared MLP (SI split 8
ways). On device each core:
  1. computes gate logits in exact fp32 (reproduces the reference top-6
     bit-exactly), sigmoid scores, top-6 + renormalized combine weights,
  2. compacts per-slot routed token lists with gpsimd sparse_gather; pad
     slots get idx 0 for the x-gather (safe read) and idx -1 for the
     scatter/cmb-gather (skipped by the DMA ucode),
  3. transpose-gathers routed token activations (bf16) straight into
     [128, H/128, slots] layout, runs the expert SwiGLU FFN on the PE array
     in bf16 (fp32 PSUM), scales by the fp32 combine weight, and scatter-ADDs
     fp32 into outp (zero-initialized by the runtime),
  4. writes its shared-MLP partial (bf16 matmuls) to a separate bf16 output.
The host sums outp + outsh over the 8 cores in fp64.
"""

import numpy as np

T, H, E, K, I, SI = 2048, 1024, 64, 6, 512, 1024
NCORE = 8
EC = E // NCORE          # expert slots per core
CAPS = [384] + [256] * 7 # per-slot token capacity (multiples of 128)
CMAX = max(CAPS)
KC = H // 128            # hidden-dim 128-chunks
ICN = I // 128           # expert-intermediate 128-chunks
TCN = T // 128           # token 128-chunks
SIC = SI // NCORE        # shared-intermediate slice per core
BIG = 1e30

_CACHE = {}


def _build():
    import concourse.bass as bass
    import concourse.tile as tile
    from concourse import bacc, mybir

    f32 = mybir.dt.float32
    bf16 = mybir.dt.bfloat16
    i32 = mybir.dt.int32
    i16 = mybir.dt.int16
    u32 = mybir.dt.uint32
    AF = mybir.ActivationFunctionType
    OP = mybir.AluOpType
    AX = mybir.AxisListType

    nc = bacc.Bacc("TRN2", target_bir_lowering=False, debug=False,
                   enable_asserts=False, num_devices=NCORE)

    xT = nc.dram_tensor("xT", [H, T], f32, kind="ExternalInput").ap()
    xp = nc.dram_tensor("xp", [T + 1, H], bf16, kind="ExternalInput").ap()
    gwT = nc.dram_tensor("gwT", [H, E], f32, kind="ExternalInput").ap()
    biasr = nc.dram_tensor("biasr", [128, E], f32, kind="ExternalInput").ap()
    wg = nc.dram_tensor("wg", [EC, H, I], bf16, kind="ExternalInput").ap()
    wu = nc.dram_tensor("wu", [EC, H, I], bf16, kind="ExternalInput").ap()
    wd = nc.dram_tensor("wd", [EC, I, H], bf16, kind="ExternalInput").ap()
    wsg = nc.dram_tensor("wsg", [H, SIC], bf16, kind="ExternalInput").ap()
    wsu = nc.dram_tensor("wsu", [H, SIC], bf16, kind="ExternalInput").ap()
    wsd = nc.dram_tensor("wsd", [SIC, H], bf16, kind="ExternalInput").ap()
    tokp1 = nc.dram_tensor("tokp1", [16, T // 16], f32, kind="ExternalInput").ap()
    pos24 = nc.dram_tensor("pos24", [16, CMAX // 16], f32, kind="ExternalInput").ap()
    outp = nc.dram_tensor("outp", [T + 1, H], f32, kind="ExternalOutput").ap()
    outsh = nc.dram_tensor("outsh", [T, H], bf16, kind="ExternalOutput").ap()

    cmb_d = nc.dram_tensor("cmb_d", [T + 1, 64], f32, kind="Internal").ap()
    sel_d = nc.dram_tensor("sel_d", [T, EC], f32, kind="Internal").ap()

    with tile.TileContext(nc) as tc:
        with (
            tc.tile_pool(name="consts", bufs=1) as consts,
            tc.tile_pool(name="wpool", bufs=2) as wpool,
            tc.tile_pool(name="etmp", bufs=2) as etmp,
            tc.tile_pool(name="smalls", bufs=1) as smalls,
            tc.tile_pool(name="ps_small", bufs=4, space="PSUM") as ps_s,
            tc.tile_pool(name="ps_big", bufs=2, space="PSUM") as ps_b,
        ):
            # ---- constants ----
            tokp1_sb = consts.tile([16, T // 16], f32)
            nc.sync.dma_start(tokp1_sb[:], tokp1)
            bias_sb = consts.tile([128, E], f32)
            nc.sync.dma_start(bias_sb[:], biasr)
            pos_sb = consts.tile([16, CMAX // 16], f32)
            nc.sync.dma_start(pos_sb[:], pos24)
            ones128 = consts.tile([128, 1], f32)
            nc.vector.memset(ones128[:], 1.0)
            ones16 = consts.tile([1, 16], f32)
            nc.vector.memset(ones16[:], 1.0)

            # routed-token index tiles (pad slots: idxP -> 0, idxN -> -1)
            idxP = smalls.tile([128, EC, CMAX // 16], i16, name="idxP")
            idxN = smalls.tile([128, EC, CMAX // 16], i16, name="idxN")

            # ---- expert weight streaming (scalar HWDGE FIFO) ----
            wg_sbs, wu_sbs, wd_sbs = [], [], []
            for e in range(EC):
                wg_sb = wpool.tile([128, KC, I], bf16, tag="wg")
                nc.scalar.dma_start(wg_sb[:], wg[e].rearrange("(kc p) i -> p kc i", p=128))
                wu_sb = wpool.tile([128, KC, I], bf16, tag="wu")
                nc.scalar.dma_start(wu_sb[:], wu[e].rearrange("(kc p) i -> p kc i", p=128))
                wd_sb = wpool.tile([128, ICN, H], bf16, tag="wd")
                nc.scalar.dma_start(wd_sb[:], wd[e].rearrange("(ic p) h -> p ic h", p=128))
                wg_sbs.append(wg_sb); wu_sbs.append(wu_sb); wd_sbs.append(wd_sb)

            with (
                tc.tile_pool(name="ph1", bufs=2) as ph1,
                tc.tile_pool(name="route", bufs=1) as route,
            ):
                # gate weights first: they gate the logits critical path
                gwT_sb = ph1.tile([128, KC, E], f32, tag="gwT")
                nc.sync.dma_start(gwT_sb[:], gwT.rearrange("(kc p) e -> p kc e", p=128))
                wsg_sb = ph1.tile([128, KC, SIC], bf16, tag="wsg")
                nc.sync.dma_start(wsg_sb[:], wsg.rearrange("(kc p) s -> p kc s", p=128))
                wsu_sb = ph1.tile([128, KC, SIC], bf16, tag="wsu")
                nc.sync.dma_start(wsu_sb[:], wsu.rearrange("(kc p) s -> p kc s", p=128))
                wsd_sb = ph1.tile([128, H], bf16, tag="wsd")
                nc.sync.dma_start(wsd_sb[:], wsd)

                scores = route.tile([128, TCN, E], f32, tag="scores")
                a_s = route.tile([128, 8, 256], bf16, tag="a_s")

                # ---- phase 1: per 256-token slab: exact-fp32 gate logits,
                # bf16 convert, shared gate/up (bf16) ----
                for sl in range(8):
                    xtl = ph1.tile([128, KC, 256], f32, tag="xtl")
                    nc.sync.dma_start(
                        xtl[:], xT.rearrange("(kc p) t -> p kc t", p=128)[:, :, sl * 256:(sl + 1) * 256])
                    for j in range(2):
                        tci = sl * 2 + j
                        pl = ps_s.tile([128, 512], f32, tag="mm_small")
                        for kc in range(KC):
                            nc.tensor.matmul(pl[:, :E], xtl[:, kc, j * 128:(j + 1) * 128],
                                             gwT_sb[:, kc, :], start=(kc == 0), stop=(kc == KC - 1))
                        nc.scalar.activation(scores[:, tci, :], pl[:, :E], AF.Sigmoid)
                    xbf = ph1.tile([128, KC, 256], bf16, tag="xbf")
                    nc.vector.tensor_copy(xbf[:], xtl[:])
                    pg = ps_s.tile([128, 512], f32, tag="mm_small")
                    pu = ps_s.tile([128, 512], f32, tag="mm_small")
                    for kc in range(KC):
                        nc.tensor.matmul(pg[:, :256], wsg_sb[:, kc, :], xbf[:, kc, :],
                                         start=(kc == 0), stop=(kc == KC - 1))
                    for kc in range(KC):
                        nc.tensor.matmul(pu[:, :256], wsu_sb[:, kc, :], xbf[:, kc, :],
                                         start=(kc == 0), stop=(kc == KC - 1))
                    sg_t = route.tile([128, 256], f32, tag="sgt")
                    nc.scalar.activation(sg_t[:], pg[:, :256], AF.Sigmoid)
                    gu_t = route.tile([128, 256], f32, tag="gut")
                    nc.vector.tensor_tensor(gu_t[:], sg_t[:], pg[:, :256], op=OP.mult)
                    nc.vector.tensor_tensor(a_s[:, sl, :], gu_t[:], pu[:, :256], op=OP.mult)

                # ---- phase 2: routing (DVE, fp32 exact) ----
                work_t = [route.tile([128, TCN, E], f32, tag=f"work{i}", name=f"work{i}")
                          for i in range(2)]
                nc.vector.tensor_tensor(
                    work_t[0][:], scores[:],
                    bias_sb[:, None, :].to_broadcast([128, TCN, E]), op=OP.add)
                wsrc = work_t[0]
                for k in range(K):
                    m = route.tile([128, TCN], f32, tag=f"m{k % 2}")
                    nc.vector.reduce_max(m[:], wsrc[:], axis=AX.X)
                    eq = route.tile([128, TCN, E], f32, tag="eq")
                    nc.vector.tensor_tensor(
                        eq[:], wsrc[:], m[:, :, None].to_broadcast([128, TCN, E]),
                        op=OP.is_equal)
                    wdst = work_t[(k + 1) % 2] if k < K - 1 else work_t[0]
                    nc.vector.scalar_tensor_tensor(
                        wdst[:], eq[:], -BIG, wsrc[:], op0=OP.mult, op1=OP.add)
                    wsrc = wdst
                sel = route.tile([128, TCN, E], f32, tag="eq")
                nc.vector.tensor_scalar(sel[:], wsrc[:], -BIG / 2, None, op0=OP.is_lt)
                selprod = route.tile([128, TCN, E], f32, tag="work1")
                nc.vector.tensor_tensor(selprod[:], scores[:], sel[:], op=OP.mult)
                denom = route.tile([128, TCN], f32, tag="denom")
                nc.vector.tensor_reduce(denom[:], selprod[:], axis=AX.X, op=OP.add)
                rec = route.tile([128, TCN], f32, tag="rec")
                nc.vector.reciprocal(rec[:], denom[:])
                cmb8 = route.tile([128, TCN, EC], f32, tag="cmb8")
                nc.vector.tensor_tensor(
                    cmb8[:], selprod[:, :, 0:EC],
                    rec[:, :, None].to_broadcast([128, TCN, EC]), op=OP.mult)

                # roundtrip through DRAM to re-wrap layouts (full 64-wide rows
                # so the 256B-row combine gather reads aligned data)
                cmbw = route.tile([128, TCN, 64], f32, tag="work1")
                nc.vector.memset(cmbw[:], 0.0)
                nc.vector.tensor_copy(cmbw[:, :, 0:EC], cmb8[:])
                nc.sync.dma_start(
                    cmb_d[0:T].rearrange("(tc p) e -> p tc e", p=128), cmbw[:])
                zrow = route.tile([1, 64], f32, tag="zrow")
                nc.vector.memset(zrow[:], 0.0)
                nc.sync.dma_start(cmb_d[T:T + 1, :], zrow[:])
                nc.sync.dma_start(sel_d.rearrange("(tc p) e -> p tc e", p=128),
                                  sel[:, :, 0:EC])
                sel16 = route.tile([16, EC, T // 16], f32, tag="sel16")
                nc.sync.dma_start(sel16[:], sel_d.rearrange("(f q) e -> q e f", q=16))

                # per-slot routed counts: ones^T @ sel8 (PE), then reduce + bcast
                pc = ps_s.tile([1, 512], f32, tag="mm_small", name="pc")
                nc.tensor.matmul(pc[0:1, 0:128], ones128[:],
                                 sel[:, :, 0:EC].rearrange("p t e -> p e t"),
                                 start=True, stop=True)
                counts = route.tile([1, EC], f32, tag="counts")
                nc.vector.tensor_reduce(counts[:], pc[0:1, 0:128].rearrange(
                    "p (e t) -> p e t", e=EC), axis=AX.X, op=OP.add)
                pnf = ps_s.tile([16, 512], f32, tag="mm_small", name="pnf")
                nc.tensor.matmul(pnf[:, 0:EC], ones16[:], counts[:],
                                 start=True, stop=True)
                nf16 = route.tile([16, EC], f32, tag="nf16")
                nc.vector.tensor_copy(nf16[:], pnf[:, 0:EC])

                # masked token values in wrapped layout: sel*(tok+1)-1
                nc.vector.tensor_tensor(
                    sel16[:], sel16[:],
                    tokp1_sb[:, None, :].to_broadcast([16, EC, T // 16]), op=OP.mult)
                nc.vector.tensor_scalar_sub(sel16[:], sel16[:], 1.0)

                # ---- compaction: per-slot routed token lists (batched mask) ----
                nfs = route.tile([1, EC], u32, tag="nfs")
                idxf = route.tile([16, EC, CMAX // 16], f32, tag="idxf")
                nc.vector.memset(idxf[:], 0.0)
                for e in range(EC):
                    nc.gpsimd.sparse_gather(idxf[:, e, 0:CAPS[e] // 16], sel16[:, e, :],
                                            num_found=nfs[0:1, e:e + 1])
                keep = route.tile([16, EC, CMAX // 16], f32, tag="keep")
                nc.vector.tensor_tensor(
                    keep[:], pos_sb[:, None, :].to_broadcast([16, EC, CMAX // 16]),
                    nf16[:, :, None].to_broadcast([16, EC, CMAX // 16]), op=OP.is_lt)
                k32 = route.tile([16, EC, CMAX // 16], i32, tag="k32")
                nc.vector.tensor_copy(k32[:], keep[:])
                km = route.tile([16, EC, CMAX // 16], i32, tag="km")
                nc.vector.tensor_scalar_mul(km[:], k32[:], -1)
                bits = route.tile([16, EC, CMAX // 16], i32, tag="bits")
                nc.vector.tensor_tensor(bits[:], idxf[:].bitcast(i32), km[:],
                                        op=OP.bitwise_and)
                km1 = route.tile([16, EC, CMAX // 16], f32, tag="km1")
                nc.vector.tensor_scalar_sub(km1[:], keep[:], 1.0)
                idxnf = route.tile([16, EC, CMAX // 16], f32, tag="idxnf")
                nc.vector.scalar_tensor_tensor(idxnf[:], km1[:], -float(T),
                                               bits[:].bitcast(f32),
                                               op0=OP.mult, op1=OP.add)
                nc.vector.tensor_copy(idxN[0:16], idxnf[:])
                nc.vector.tensor_copy(idxP[0:16], idxnf[:])
                for t_ in (idxP, idxN):
                    nc.sync.dma_start(t_[16:32], t_[0:16])
                    nc.sync.dma_start(t_[32:64], t_[0:32])
                    nc.sync.dma_start(t_[64:128], t_[0:64])

                # ---- phase 3: shared down-proj (bf16), PE fills the routing
                # latency; dense bf16 write of the shared partial ----
                for tci in range(TCN):
                    sl, j = tci // 2, tci % 2
                    py = ps_b.tile([128, H], f32, tag="mm_big")
                    for nh in range(2):
                        nc.tensor.matmul(py[:, nh * 512:(nh + 1) * 512],
                                         a_s[:, sl, j * 128:(j + 1) * 128],
                                         wsd_sb[:, nh * 512:(nh + 1) * 512],
                                         start=True, stop=True)
                    ysh = route.tile([128, H], bf16, tag="ysh", bufs=2)
                    nc.scalar.activation(ysh[:, 0:512], py[:, 0:512], AF.Copy)
                    nc.vector.tensor_copy(ysh[:, 512:1024], py[:, 512:1024])
                    nc.sync.dma_start(
                        outsh.rearrange("(tc p) h -> p tc h", p=128)[:, tci, :], ysh[:])

            # ---- phase 4: expert loop (software-pipelined: slot e+1's
            # gathers are issued on the soft-DMA queue BEFORE slot e's
            # scatter, so the prefetch is never head-of-line blocked) ----
            with tc.tile_pool(name="xpool", bufs=2) as xpool:
                xgs, cgs = {}, {}

                def emit_gathers(e):
                    C = CAPS[e]
                    xg = xpool.tile([128, KC, C], bf16, tag=f"xe{C}", name=f"xg{e}")
                    nc.gpsimd.dma_gather(xg[:], xp, idxP[:, e, 0:C // 16], C, C, H,
                                         transpose=True)
                    cg = xpool.tile([128, C // 128, 64], f32, tag=f"cg{C}", name=f"cg{e}")
                    nc.gpsimd.dma_gather(cg[:], cmb_d, idxP[:, e, 0:C // 16], C, C, 64)
                    xgs[e], cgs[e] = xg, cg

                emit_gathers(0)
                for e in range(EC):
                    C = CAPS[e]
                    if e + 1 < EC:
                        emit_gathers(e + 1)
                    wg_sb, wu_sb, wd_sb = wg_sbs[e], wu_sbs[e], wd_sbs[e]
                    xeT, cg = xgs.pop(e), cgs.pop(e)

                    aT = xpool.tile([128, ICN, C], bf16, tag=f"aT{C}")
                    for ic in range(ICN):
                        pg = ps_s.tile([128, 512], f32, tag="mm_small")
                        pu = ps_s.tile([128, 512], f32, tag="mm_small")
                        for kc in range(KC):
                            nc.tensor.matmul(pg[:, :C], wg_sb[:, kc, ic * 128:(ic + 1) * 128],
                                             xeT[:, kc, :], start=(kc == 0), stop=(kc == KC - 1))
                        for kc in range(KC):
                            nc.tensor.matmul(pu[:, :C], wu_sb[:, kc, ic * 128:(ic + 1) * 128],
                                             xeT[:, kc, :], start=(kc == 0), stop=(kc == KC - 1))
                        sg_t = etmp.tile([128, CMAX], f32, tag="esilu")
                        nc.scalar.activation(sg_t[:, :C], pg[:, :C], AF.Sigmoid)
                        gu_t = etmp.tile([128, CMAX], f32, tag="egu")
                        nc.vector.tensor_tensor(gu_t[:, :C], sg_t[:, :C], pg[:, :C], op=OP.mult)
                        nc.vector.tensor_tensor(aT[:, ic, :], gu_t[:, :C], pu[:, :C], op=OP.mult)

                    y_sb = xpool.tile([128, C // 128, H], f32, tag=f"y{C}")
                    for cc in range(C // 128):
                        py = ps_b.tile([128, H], f32, tag="mm_big")
                        for ic in range(ICN):
                            for nh in range(2):
                                nc.tensor.matmul(py[:, nh * 512:(nh + 1) * 512],
                                                 aT[:, ic, cc * 128:(cc + 1) * 128],
                                                 wd_sb[:, ic, nh * 512:(nh + 1) * 512],
                                                 start=(ic == 0), stop=(ic == ICN - 1))
                        nc.scalar.activation(y_sb[:, cc, :], py[:], AF.Copy,
                                             scale=cg[:, cc, e:e + 1])
                    nc.gpsimd.dma_scatter_add(outp, y_sb[:], idxN[:, e, 0:C // 16],
                                              C, C, H)

    nc.compile()
    return nc


def _route_counts(x, gate_w, gate_bias):
    """Host-side routing counts (fp64) for load-balanced expert assignment."""
    logits = x.astype(np.float64) @ gate_w.astype(np.float64).T
    scores = 1.0 / (1.0 + np.exp(-logits))
    idx = np.argsort(-(scores + gate_bias.astype(np.float64)), axis=1)[:, :K]
    return np.bincount(idx.ravel(), minlength=E)


def _prep_in_maps(inputs):
    import ml_dtypes
    bf = ml_dtypes.bfloat16
    x = np.ascontiguousarray(inputs["hidden_states"], dtype=np.float32)
    gate_w = np.asarray(inputs["gate_w"], dtype=np.float32)
    gate_bias = np.asarray(inputs["gate_bias"], dtype=np.float32)
    w_gate = np.asarray(inputs["w_gate"], dtype=np.float32)
    w_up = np.asarray(inputs["w_up"], dtype=np.float32)
    w_down = np.asarray(inputs["w_down"], dtype=np.float32)
    ws_gate = np.asarray(inputs["ws_gate"], dtype=np.float32)
    ws_up = np.asarray(inputs["ws_up"], dtype=np.float32)
    ws_down = np.asarray(inputs["ws_down"], dtype=np.float32)

    # snake assignment: slot s takes count-ranks [8s, 8s+8), alternating core
    # order, so per-slot capacity needs are uniform across cores and each core
    # gets a balanced token total.
    counts = _route_counts(x, gate_w, gate_bias)
    order = np.argsort(-counts, kind="stable")
    perm = np.zeros((NCORE, EC), dtype=np.int64)
    for s in range(EC):
        band = order[8 * s:8 * s + 8]
        perm[:, s] = band if s % 2 == 0 else band[::-1]
        assert counts[band].max() + 8 < CAPS[s], (s, counts[band].max(), CAPS[s])

    xTc = np.ascontiguousarray(x.T)
    xbf = x.astype(bf)
    tokp1 = (np.arange(16)[:, None] + 16 * np.arange(T // 16)[None, :] + 1).astype(np.float32)
    pos24 = (np.arange(16)[:, None] + 16 * np.arange(CMAX // 16)[None, :]).astype(np.float32)

    in_maps = []
    for c in range(NCORE):
        loc = list(perm[c])
        gorder = loc + [e for e in range(E) if e not in loc]
        in_maps.append({
            "xp": np.vstack([xbf, np.zeros((1, H), bf)]),
            "xT": xTc,
            "gwT": np.ascontiguousarray(gate_w[gorder].T),
            "biasr": np.ascontiguousarray(
                np.broadcast_to(gate_bias[0, gorder], (128, E))).astype(np.float32),
            "wg": np.ascontiguousarray(w_gate[loc]).astype(bf),
            "wu": np.ascontiguousarray(w_up[loc]).astype(bf),
            "wd": np.ascontiguousarray(w_down[loc]).astype(bf),
            "wsg": np.ascontiguousarray(ws_gate[:, c * SIC:(c + 1) * SIC]).astype(bf),
            "wsu": np.ascontiguousarray(ws_up[:, c * SIC:(c + 1) * SIC]).astype(bf),
            "wsd": np.ascontiguousarray(ws_down[c * SIC:(c + 1) * SIC, :]).astype(bf),
            "tokp1": tokp1,
            "pos24": pos24,
        })
    return in_maps


def get_nc():
    if "nc" not in _CACHE:
        _CACHE["nc"] = _build()
    return _CACHE["nc"]


def kernel(**inputs) -> np.ndarray:
    from concourse import bass_utils
    nc = get_nc()
    in_maps = _prep_in_maps(inputs)
    res = bass_utils.run_bass_kernel_spmd(nc, in_maps, core_ids=list(range(NCORE)))
    acc = np.zeros((T, H), dtype=np.float64)
    for c in range(NCORE):
        acc += res.results[c]["outp"][0:T].astype(np.float64)
        acc += res.results[c]["outsh"].astype(np.float64)
    return acc.astype(np.float32)
